# revision 6
# baseline (speedup 1.0000x reference)
"""Trainium2 Bass kernel for nn_MHAttention_60215441490286.

Reference computation (per batch b=2, seq s=2048, E=1024, H=16, D=64):
    q = x @ Wq.T + bq   (same k, v)
    q -> reshape(b, s, h, d) -> reshape(-1, s, d)      # RAW reshape, no transpose
    qk = einsum('bqd,bkd->bqk', q, k) / sqrt(d)
    out = einsum('bqk,bkd->bqd', qk, v)                # NO softmax

Key structure facts exploited here:
  * The raw reshape means slice i of the (32, 2048, 64) tensors is just rows
    [128*i, 128*(i+1)) of the flat (4096, 1024) projection output, reshaped.
    So slice i depends only on 128 rows of x -> embarrassingly parallel over
    slices. 8 cores x 4 slices each, no collectives.
  * No softmax => O_i = (Q_i K_i^T) V_i = Q_i (K_i^T V_i). The inner
    M_i = K_i^T V_i is only 64x64, so the 2048x2048 score matrix is never
    materialized. Attention FLOPs drop ~32x; QKV projections dominate.
  * All matmuls run as float32r (full PE rate at N>=512, fp32 memory layout).
  * scale 1/sqrt(64)=1/8 is folded into Wq/bq on the host.

Per-core layouts (core c owns x rows [512c, 512c+512) = slices 4c..4c+3):
  xt   (1024, 512)  = x rows transposed (e on partitions)
  wqt/wkt/wvt (1024,1024) = W.T (e on partitions)
  Q: computed as Q^T tiles (m on partitions, r free), then reorganized into
     qTi (64, 8192) interleaved layout: qTi[dd, s*2048 + r1*16 + c1]
     = Qraw[512c + s*128 + r1, 64*c1 + dd]  (this is Q_slice^T per slice)
  K,V: computed in natural orientation knat/vnat[s] (128 rows, 1024 cols)
  M_s = sum_c2 Kc2^T @ Vc2 (64x64 in PSUM, 16 accumulating matmuls)
  O_s^T = M_s^T @ Q_s^T -> matmul(lhsT=M_s, rhs=qTi slice) (64, 2048)
  out DRAM per core: (4, 64, 2048) = O^T per slice; host transposes.
"""

import sys

if "/opt/trn_rl_repo" not in sys.path:
    sys.path.insert(0, "/opt/trn_rl_repo")

from contextlib import ExitStack

import numpy as np

import concourse.bass as bass
import concourse.tile as tile
from concourse import bacc
from concourse import mybir
from concourse.bass_utils import run_bass_kernel_spmd

F32 = mybir.dt.float32
F32R = mybir.dt.float32r
BF16 = mybir.dt.bfloat16

NCORES = 8
E = 1024
D = 64
H = 16
RPC = 512  # x rows per core
SLICES = 4  # slices per core (32 total / 8 cores)
SEQ = 2048  # positions per slice (128 rows x 16 chunks)

# test.py pokes these
TRACE = False
LAST_RESULT = None
_BUILT = None


def _body(ctx: ExitStack, tc: tile.TileContext, d):
    nc = tc.nc

    const = ctx.enter_context(tc.tile_pool(name="const", bufs=1))
    xpool = ctx.enter_context(tc.tile_pool(name="xp", bufs=1))
    wpool = ctx.enter_context(tc.tile_pool(name="wp", bufs=6))
    actp = ctx.enter_context(tc.tile_pool(name="act", bufs=1))
    smallp = ctx.enter_context(tc.tile_pool(name="small", bufs=2))
    opool = ctx.enter_context(tc.tile_pool(name="osb", bufs=2))
    psp = ctx.enter_context(tc.tile_pool(name="psp", bufs=5, space="PSUM"))
    psm = ctx.enter_context(tc.tile_pool(name="psm", bufs=1, space="PSUM"))
    pso = ctx.enter_context(tc.tile_pool(name="pso", bufs=2, space="PSUM"))

    # --- constants ---
    ones = const.tile([1, RPC], BF16, name="ones")
    nc.any.memset(ones, 1.0)
    bq_s = const.tile([1, E], BF16, name="bq_s")
    nc.sync.dma_start(bq_s, d["bq"])
    bk_s = const.tile([1, E], BF16, name="bk_s")
    nc.sync.dma_start(bk_s, d["bk"])
    bv_s = const.tile([1, E], BF16, name="bv_s")
    nc.sync.dma_start(bv_s, d["bv"])

    # --- x^T tiles: 8 x (128, 512) ---
    xt_sb = []
    for kt in range(8):
        t = xpool.tile([128, RPC], BF16, name=f"xt{kt}")
        nc.sync.dma_start(t, d["xt"][kt * 128 : (kt + 1) * 128, :])
        xt_sb.append(t)

    # --- activation destinations ---
    # qTi[dd, s*2048 + r1*16 + c1] ; view (64, 4, 128, 16)
    qTi = actp.tile([64, SLICES * SEQ], BF16, name="qTi")
    qTi_v = qTi.rearrange("d (s r c) -> d s r c", s=SLICES, r=128, c=16)
    knat = [actp.tile([128, E], BF16, name=f"knat{s}") for s in range(SLICES)]
    vnat = [actp.tile([128, E], BF16, name=f"vnat{s}") for s in range(SLICES)]

    # --- Q projection: out Q^T tiles (m on partitions, r free) ---
    # For each n-chunk (512 of m), 4 m-tiles of 128 accumulate over kt.
    for nch in range(2):
        ps_q = [psp.tile([128, RPC], F32, tag="proj", name=f"psq{nch}_{i}") for i in range(4)]
        for m4 in range(4):
            mt = nch * 4 + m4
            nc.tensor.matmul(
                ps_q[m4],
                bq_s[:, mt * 128 : (mt + 1) * 128],
                ones,
                start=True,
                stop=False,
            )
        for kt in range(8):
            w = wpool.tile([128, RPC], BF16, tag="w")
            nc.sync.dma_start(
                w, d["wqt"][kt * 128 : (kt + 1) * 128, nch * 512 : (nch + 1) * 512]
            )
            for m4 in range(4):
                nc.tensor.matmul(
                    ps_q[m4],
                    w[:, m4 * 128 : (m4 + 1) * 128],
                    xt_sb[kt],
                    start=False,
                    stop=(kt == 7),
                )
        # reorganize into qTi: psum (m=(c1,dd) x r=(s,r1)) -> qTi[dd, s, r1, c1]
        for m4 in range(4):
            mt = nch * 4 + m4
            for h in range(2):
                c1 = 2 * mt + h
                src = ps_q[m4][h * 64 : (h + 1) * 64, :].rearrange(
                    "d (s r) -> d s r", s=SLICES
                )
                dst = qTi_v[:, :, :, c1]
                nc.vector.tensor_copy(dst, src)

    # --- K and V projections: natural orientation (r on partitions) ---
    for name_w, bias_t, dest in (("wkt", bk_s, knat), ("wvt", bv_s, vnat)):
        for nch in range(2):
            ps_kv = [psp.tile([128, RPC], F32, tag="proj", name=f"pskv{name_w}{nch}_{i}") for i in range(4)]
            for rt in range(4):
                nc.tensor.matmul(
                    ps_kv[rt],
                    ones[:, :128],
                    bias_t[:, nch * 512 : (nch + 1) * 512],
                    start=True,
                    stop=False,
                )
            for kt in range(8):
                w = wpool.tile([128, RPC], BF16, tag="w")
                nc.sync.dma_start(
                    w,
                    d[name_w][kt * 128 : (kt + 1) * 128, nch * 512 : (nch + 1) * 512],
                )
                for rt in range(4):
                    nc.tensor.matmul(
                        ps_kv[rt],
                        xt_sb[kt][:, rt * 128 : (rt + 1) * 128],
                        w,
                        start=False,
                        stop=(kt == 7),
                    )
            for rt in range(4):
                dst = dest[rt][:, nch * 512 : (nch + 1) * 512]
                nc.vector.tensor_copy(dst, ps_kv[rt])

    # --- attention: M_s = sum_c2 Kc2^T Vc2 ; O_s^T = M_s^T Q_s^T ---
    for s in range(SLICES):
        ps_m = psm.tile([64, 64], F32, tag="m")
        for c2 in range(16):
            nc.tensor.matmul(
                ps_m,
                knat[s][:, c2 * 64 : (c2 + 1) * 64],
                vnat[s][:, c2 * 64 : (c2 + 1) * 64],
                start=(c2 == 0),
                stop=(c2 == 15),
            )
        m_sb = smallp.tile([64, 64], BF16, tag="msb")
        nc.vector.tensor_copy(m_sb, ps_m)

        ot_sb = opool.tile([64, SEQ], F32, tag="ot")
        for g in range(4):
            ps_ot = pso.tile([64, 512], F32, tag="pot")
            nc.tensor.matmul(
                ps_ot,
                m_sb,
                qTi[:, (s * 4 + g) * 512 : (s * 4 + g + 1) * 512],
                start=True,
                stop=True,
            )
            dst = ot_sb[:, g * 512 : (g + 1) * 512]
            if g % 2 == 0:
                nc.vector.tensor_copy(dst, ps_ot)
            else:
                nc.scalar.copy(dst, ps_ot)
        nc.sync.dma_start(d["out"][s], ot_sb)


def _build():
    global _BUILT
    if _BUILT is not None:
        return _BUILT
    nc = bacc.Bacc(trn_type="TRN2", target_bir_lowering=False, debug=False)
    d = {
        "xt": nc.dram_tensor("xt", [E, RPC], BF16, kind="ExternalInput").ap(),
        "wqt": nc.dram_tensor("wqt", [E, E], BF16, kind="ExternalInput").ap(),
        "wkt": nc.dram_tensor("wkt", [E, E], BF16, kind="ExternalInput").ap(),
        "wvt": nc.dram_tensor("wvt", [E, E], BF16, kind="ExternalInput").ap(),
        "bq": nc.dram_tensor("bq", [1, E], BF16, kind="ExternalInput").ap(),
        "bk": nc.dram_tensor("bk", [1, E], BF16, kind="ExternalInput").ap(),
        "bv": nc.dram_tensor("bv", [1, E], BF16, kind="ExternalInput").ap(),
        "out": nc.dram_tensor("out", [SLICES, D, SEQ], F32, kind="ExternalOutput").ap(),
    }
    with tile.TileContext(nc) as tc:
        with ExitStack() as ctx:
            _body(ctx, tc, d)
    nc.compile()
    _BUILT = nc
    return nc


def kernel(x, Wq, bq, Wk, bk, Wv, bv):
    global LAST_RESULT
    x = np.asarray(x, dtype=np.float32)
    scale = np.float32(1.0 / 8.0)

    import ml_dtypes

    bf16 = ml_dtypes.bfloat16
    wqt = (np.ascontiguousarray(np.asarray(Wq, np.float32).T) * scale).astype(bf16)
    wkt = np.ascontiguousarray(np.asarray(Wk, np.float32).T).astype(bf16)
    wvt = np.ascontiguousarray(np.asarray(Wv, np.float32).T).astype(bf16)
    bq2 = (np.asarray(bq, np.float32) * scale).reshape(1, E).astype(bf16)
    bk2 = np.asarray(bk, np.float32).reshape(1, E).astype(bf16)
    bv2 = np.asarray(bv, np.float32).reshape(1, E).astype(bf16)

    x2 = x.reshape(-1, E)  # (4096, 1024)
    in_maps = []
    for c in range(NCORES):
        xt = np.ascontiguousarray(x2[c * RPC : (c + 1) * RPC].T).astype(bf16)
        in_maps.append(
            {
                "xt": xt,
                "wqt": wqt,
                "wkt": wkt,
                "wvt": wvt,
                "bq": bq2,
                "bk": bk2,
                "bv": bv2,
            }
        )

    nc = _build()
    res = run_bass_kernel_spmd(nc, in_maps, core_ids=list(range(NCORES)), trace=TRACE)
    LAST_RESULT = res

    out = np.empty((NCORES * SLICES, SEQ, D), dtype=np.float32)
    for c in range(NCORES):
        oc = res.results[c]["out"]  # (4, 64, 2048)
        for s in range(SLICES):
            out[c * SLICES + s] = oc[s].T
    return out


# revision 7
# speedup vs baseline: 1.1197x; 1.1197x over previous
"""Trainium2 Bass kernel for nn_MHAttention_60215441490286.

Reference computation (per batch b=2, seq s=2048, E=1024, H=16, D=64):
    q = x @ Wq.T + bq   (same k, v)
    q -> reshape(b, s, h, d) -> reshape(-1, s, d)      # RAW reshape, no transpose
    qk = einsum('bqd,bkd->bqk', q, k) / sqrt(d)
    out = einsum('bqk,bkd->bqd', qk, v)                # NO softmax

Key structure facts exploited here:
  * The raw reshape means slice i of the (32, 2048, 64) tensors is just rows
    [128*i, 128*(i+1)) of the flat (4096, 1024) projection output, reshaped.
    So slice i depends only on 128 rows of x -> embarrassingly parallel over
    slices. 8 cores x 4 slices each, no collectives.
  * No softmax => O_i = (Q_i K_i^T) V_i = Q_i (K_i^T V_i). The inner
    M_i = K_i^T V_i is only 64x64, so the 2048x2048 score matrix is never
    materialized. Attention FLOPs drop ~32x; QKV projections dominate.
  * All matmuls run as float32r (full PE rate at N>=512, fp32 memory layout).
  * scale 1/sqrt(64)=1/8 is folded into Wq/bq on the host.

Per-core layouts (core c owns x rows [512c, 512c+512) = slices 4c..4c+3):
  xt   (1024, 512)  = x rows transposed (e on partitions)
  wqt/wkt/wvt (1024,1024) = W.T (e on partitions)
  Q: computed as Q^T tiles (m on partitions, r free), then reorganized into
     qTi (64, 8192) interleaved layout: qTi[dd, s*2048 + r1*16 + c1]
     = Qraw[512c + s*128 + r1, 64*c1 + dd]  (this is Q_slice^T per slice)
  K,V: computed in natural orientation knat/vnat[s] (128 rows, 1024 cols)
  M_s = sum_c2 Kc2^T @ Vc2 (64x64 in PSUM, 16 accumulating matmuls)
  O_s^T = M_s^T @ Q_s^T -> matmul(lhsT=M_s, rhs=qTi slice) (64, 2048)
  out DRAM per core: (4, 64, 2048) = O^T per slice; host transposes.
"""

import sys

if "/opt/trn_rl_repo" not in sys.path:
    sys.path.insert(0, "/opt/trn_rl_repo")

from contextlib import ExitStack

import numpy as np

import concourse.bass as bass
import concourse.tile as tile
from concourse import bacc
from concourse import mybir
from concourse.bass_utils import run_bass_kernel_spmd

F32 = mybir.dt.float32
F32R = mybir.dt.float32r
BF16 = mybir.dt.bfloat16

NCORES = 8
E = 1024
D = 64
H = 16
RPC = 512  # x rows per core
SLICES = 4  # slices per core (32 total / 8 cores)
SEQ = 2048  # positions per slice (128 rows x 16 chunks)

# test.py pokes these
TRACE = False
LAST_RESULT = None
_BUILT = None


def _body(ctx: ExitStack, tc: tile.TileContext, d):
    nc = tc.nc

    const = ctx.enter_context(tc.tile_pool(name="const", bufs=1))
    xpool = ctx.enter_context(tc.tile_pool(name="xp", bufs=1))
    wpool = ctx.enter_context(tc.tile_pool(name="wp", bufs=12))
    actp = ctx.enter_context(tc.tile_pool(name="act", bufs=1))
    smallp = ctx.enter_context(tc.tile_pool(name="small", bufs=2))
    opool = ctx.enter_context(tc.tile_pool(name="osb", bufs=2))
    psp = ctx.enter_context(tc.tile_pool(name="psp", bufs=5, space="PSUM"))
    psm = ctx.enter_context(tc.tile_pool(name="psm", bufs=1, space="PSUM"))
    pso = ctx.enter_context(tc.tile_pool(name="pso", bufs=2, space="PSUM"))

    # --- constants ---
    ones = const.tile([1, RPC], BF16, name="ones")
    nc.any.memset(ones, 1.0)
    bq_s = const.tile([1, E], BF16, name="bq_s")
    nc.sync.dma_start(bq_s, d["bq"])
    bk_s = const.tile([1, E], BF16, name="bk_s")
    nc.sync.dma_start(bk_s, d["bk"])
    bv_s = const.tile([1, E], BF16, name="bv_s")
    nc.sync.dma_start(bv_s, d["bv"])

    # --- x^T tiles: 8 x (128, 512) ---
    xt_sb = []
    for kt in range(8):
        t = xpool.tile([128, RPC], BF16, name=f"xt{kt}")
        nc.sync.dma_start(t, d["xt"][kt * 128 : (kt + 1) * 128, :])
        xt_sb.append(t)

    # --- activation destinations ---
    # qTi[dd, s*2048 + r1*16 + c1] ; view (64, 4, 128, 16)
    qTi = actp.tile([64, SLICES * SEQ], BF16, name="qTi")
    qTi_v = qTi.rearrange("d (s r c) -> d s r c", s=SLICES, r=128, c=16)
    knat = [actp.tile([128, E], BF16, name=f"knat{s}") for s in range(SLICES)]
    vnat = [actp.tile([128, E], BF16, name=f"vnat{s}") for s in range(SLICES)]

    # --- Q projection: out Q^T tiles (m on partitions, r free) ---
    # For each n-chunk (512 of m), 4 m-tiles of 128 accumulate over kt.
    for nch in range(2):
        ps_q = [psp.tile([128, RPC], F32, tag="proj", name=f"psq{nch}_{i}") for i in range(4)]
        for m4 in range(4):
            mt = nch * 4 + m4
            nc.tensor.matmul(
                ps_q[m4],
                bq_s[:, mt * 128 : (mt + 1) * 128],
                ones,
                start=True,
                stop=False,
            )
        for kt in range(8):
            w = wpool.tile([128, RPC], BF16, tag="w")
            nc.sync.dma_start(
                w, d["wqt"][kt * 128 : (kt + 1) * 128, nch * 512 : (nch + 1) * 512]
            )
            for m4 in range(4):
                nc.tensor.matmul(
                    ps_q[m4],
                    w[:, m4 * 128 : (m4 + 1) * 128],
                    xt_sb[kt],
                    start=False,
                    stop=(kt == 7),
                )
        # reorganize into qTi: psum (m=(c1,dd) x r=(s,r1)) -> qTi[dd, s, r1, c1]
        for m4 in range(4):
            mt = nch * 4 + m4
            for h in range(2):
                c1 = 2 * mt + h
                src = ps_q[m4][h * 64 : (h + 1) * 64, :].rearrange(
                    "d (s r) -> d s r", s=SLICES
                )
                dst = qTi_v[:, :, :, c1]
                if h == 0:
                    nc.vector.tensor_copy(dst, src)
                else:
                    nc.scalar.copy(dst, src)

    # --- K and V projections: natural orientation (r on partitions) ---
    for name_w, bias_t, dest in (("wkt", bk_s, knat), ("wvt", bv_s, vnat)):
        for nch in range(2):
            ps_kv = [psp.tile([128, RPC], F32, tag="proj", name=f"pskv{name_w}{nch}_{i}") for i in range(4)]
            for rt in range(4):
                nc.tensor.matmul(
                    ps_kv[rt],
                    ones[:, :128],
                    bias_t[:, nch * 512 : (nch + 1) * 512],
                    start=True,
                    stop=False,
                )
            for kt in range(8):
                w = wpool.tile([128, RPC], BF16, tag="w")
                nc.sync.dma_start(
                    w,
                    d[name_w][kt * 128 : (kt + 1) * 128, nch * 512 : (nch + 1) * 512],
                )
                for rt in range(4):
                    nc.tensor.matmul(
                        ps_kv[rt],
                        xt_sb[kt][:, rt * 128 : (rt + 1) * 128],
                        w,
                        start=False,
                        stop=(kt == 7),
                    )
            for rt in range(4):
                dst = dest[rt][:, nch * 512 : (nch + 1) * 512]
                if rt % 2 == 0:
                    nc.vector.tensor_copy(dst, ps_kv[rt])
                else:
                    nc.scalar.copy(dst, ps_kv[rt])

    # --- attention: M_s = sum_c2 Kc2^T Vc2 ; O_s^T = M_s^T Q_s^T ---
    for s in range(SLICES):
        ps_m = psm.tile([64, 64], F32, tag="m")
        for c2 in range(16):
            nc.tensor.matmul(
                ps_m,
                knat[s][:, c2 * 64 : (c2 + 1) * 64],
                vnat[s][:, c2 * 64 : (c2 + 1) * 64],
                start=(c2 == 0),
                stop=(c2 == 15),
            )
        m_sb = smallp.tile([64, 64], BF16, tag="msb")
        nc.vector.tensor_copy(m_sb, ps_m)

        ot_sb = opool.tile([64, SEQ], F32, tag="ot")
        for g in range(4):
            ps_ot = pso.tile([64, 512], F32, tag="pot")
            nc.tensor.matmul(
                ps_ot,
                m_sb,
                qTi[:, (s * 4 + g) * 512 : (s * 4 + g + 1) * 512],
                start=True,
                stop=True,
            )
            dst = ot_sb[:, g * 512 : (g + 1) * 512]
            if g % 2 == 0:
                nc.vector.tensor_copy(dst, ps_ot)
            else:
                nc.scalar.copy(dst, ps_ot)
        nc.sync.dma_start(d["out"][s], ot_sb)


def _build():
    global _BUILT
    if _BUILT is not None:
        return _BUILT
    nc = bacc.Bacc(trn_type="TRN2", target_bir_lowering=False, debug=False)
    d = {
        "xt": nc.dram_tensor("xt", [E, RPC], BF16, kind="ExternalInput").ap(),
        "wqt": nc.dram_tensor("wqt", [E, E], BF16, kind="ExternalInput").ap(),
        "wkt": nc.dram_tensor("wkt", [E, E], BF16, kind="ExternalInput").ap(),
        "wvt": nc.dram_tensor("wvt", [E, E], BF16, kind="ExternalInput").ap(),
        "bq": nc.dram_tensor("bq", [1, E], BF16, kind="ExternalInput").ap(),
        "bk": nc.dram_tensor("bk", [1, E], BF16, kind="ExternalInput").ap(),
        "bv": nc.dram_tensor("bv", [1, E], BF16, kind="ExternalInput").ap(),
        "out": nc.dram_tensor("out", [SLICES, D, SEQ], F32, kind="ExternalOutput").ap(),
    }
    with tile.TileContext(nc) as tc:
        with ExitStack() as ctx:
            _body(ctx, tc, d)
    nc.compile()
    _BUILT = nc
    return nc


def kernel(x, Wq, bq, Wk, bk, Wv, bv):
    global LAST_RESULT
    x = np.asarray(x, dtype=np.float32)
    scale = np.float32(1.0 / 8.0)

    import ml_dtypes

    bf16 = ml_dtypes.bfloat16
    wqt = (np.ascontiguousarray(np.asarray(Wq, np.float32).T) * scale).astype(bf16)
    wkt = np.ascontiguousarray(np.asarray(Wk, np.float32).T).astype(bf16)
    wvt = np.ascontiguousarray(np.asarray(Wv, np.float32).T).astype(bf16)
    bq2 = (np.asarray(bq, np.float32) * scale).reshape(1, E).astype(bf16)
    bk2 = np.asarray(bk, np.float32).reshape(1, E).astype(bf16)
    bv2 = np.asarray(bv, np.float32).reshape(1, E).astype(bf16)

    x2 = x.reshape(-1, E)  # (4096, 1024)
    in_maps = []
    for c in range(NCORES):
        xt = np.ascontiguousarray(x2[c * RPC : (c + 1) * RPC].T).astype(bf16)
        in_maps.append(
            {
                "xt": xt,
                "wqt": wqt,
                "wkt": wkt,
                "wvt": wvt,
                "bq": bq2,
                "bk": bk2,
                "bv": bv2,
            }
        )

    nc = _build()
    res = run_bass_kernel_spmd(nc, in_maps, core_ids=list(range(NCORES)), trace=TRACE)
    LAST_RESULT = res

    out = np.empty((NCORES * SLICES, SEQ, D), dtype=np.float32)
    for c in range(NCORES):
        oc = res.results[c]["out"]  # (4, 64, 2048)
        for s in range(SLICES):
            out[c * SLICES + s] = oc[s].T
    return out


# revision 9
# speedup vs baseline: 1.1310x; 1.0101x over previous
"""Trainium2 Bass kernel for nn_MHAttention_60215441490286.

Reference computation (per batch b=2, seq s=2048, E=1024, H=16, D=64):
    q = x @ Wq.T + bq   (same k, v)
    q -> reshape(b, s, h, d) -> reshape(-1, s, d)      # RAW reshape, no transpose
    qk = einsum('bqd,bkd->bqk', q, k) / sqrt(d)
    out = einsum('bqk,bkd->bqd', qk, v)                # NO softmax

Key structure facts exploited here:
  * The raw reshape means slice i of the (32, 2048, 64) tensors is just rows
    [128*i, 128*(i+1)) of the flat (4096, 1024) projection output, reshaped.
    So slice i depends only on 128 rows of x -> embarrassingly parallel over
    slices. 8 cores x 4 slices each, no collectives.
  * No softmax => O_i = (Q_i K_i^T) V_i = Q_i (K_i^T V_i). The inner
    M_i = K_i^T V_i is only 64x64, so the 2048x2048 score matrix is never
    materialized. Attention FLOPs drop ~32x; QKV projections dominate.
  * All matmuls run in bf16 (host-cast inputs, f32 PSUM accumulation).
  * scale 1/sqrt(64)=1/8 is folded into Wq/bq on the host.

Per-core layouts (core c owns x rows [512c, 512c+512) = slices 4c..4c+3):
  xt   (1024, 512)  = x rows transposed (e on partitions)
  wqt/wkt/wvt (1024,1024) = W.T (e on partitions)
  Q: computed as Q^T tiles (m on partitions, r free), then reorganized into
     qTi (64, 8192) interleaved layout: qTi[dd, s*2048 + r1*16 + c1]
     = Qraw[512c + s*128 + r1, 64*c1 + dd]  (this is Q_slice^T per slice)
  K,V: computed in natural orientation knat/vnat[s] (128 rows, 1024 cols)
  M_s = sum_c2 Kc2^T @ Vc2 (64x64 in PSUM, 16 accumulating matmuls)
  O_s^T = M_s^T @ Q_s^T -> matmul(lhsT=M_s, rhs=qTi slice) (64, 2048)
  out DRAM per core: (4, 64, 2048) = O^T per slice; host transposes.
"""

import sys

if "/opt/trn_rl_repo" not in sys.path:
    sys.path.insert(0, "/opt/trn_rl_repo")

from contextlib import ExitStack

import numpy as np

import concourse.bass as bass
import concourse.tile as tile
from concourse import bacc
from concourse import mybir
from concourse.bass_utils import run_bass_kernel_spmd

F32 = mybir.dt.float32
F32R = mybir.dt.float32r
BF16 = mybir.dt.bfloat16

NCORES = 8
E = 1024
D = 64
H = 16
RPC = 512  # x rows per core
SLICES = 4  # slices per core (32 total / 8 cores)
SEQ = 2048  # positions per slice (128 rows x 16 chunks)

# test.py pokes these
TRACE = False
LAST_RESULT = None
_BUILT = None


def _body(ctx: ExitStack, tc: tile.TileContext, d):
    nc = tc.nc

    const = ctx.enter_context(tc.tile_pool(name="const", bufs=1))
    xpool = ctx.enter_context(tc.tile_pool(name="xp", bufs=1))
    wpool = ctx.enter_context(tc.tile_pool(name="wp", bufs=12))
    actp = ctx.enter_context(tc.tile_pool(name="act", bufs=1))
    smallp = ctx.enter_context(tc.tile_pool(name="small", bufs=2))
    opool = ctx.enter_context(tc.tile_pool(name="osb", bufs=2))
    psp = ctx.enter_context(tc.tile_pool(name="psp", bufs=6, space="PSUM"))
    pso = ctx.enter_context(tc.tile_pool(name="pso", bufs=2, space="PSUM"))

    # --- constants ---
    ones = const.tile([1, RPC], BF16, name="ones")
    nc.any.memset(ones, 1.0)
    bq_s = const.tile([1, E], BF16, name="bq_s")
    nc.sync.dma_start(bq_s, d["bq"])
    bk_s = const.tile([1, E], BF16, name="bk_s")
    nc.sync.dma_start(bk_s, d["bk"])
    bv_s = const.tile([1, E], BF16, name="bv_s")
    nc.sync.dma_start(bv_s, d["bv"])

    # --- x^T tiles: 8 x (128, 512) ---
    xt_sb = []
    for kt in range(8):
        t = xpool.tile([128, RPC], BF16, name=f"xt{kt}")
        nc.sync.dma_start(t, d["xt"][kt * 128 : (kt + 1) * 128, :])
        xt_sb.append(t)

    # --- activation destinations ---
    # qTi[dd, s*2048 + r1*16 + c1] ; view (64, 4, 128, 16)
    qTi = actp.tile([64, SLICES * SEQ], BF16, name="qTi")
    qTi_v = qTi.rearrange("d (s r c) -> d s r c", s=SLICES, r=128, c=16)
    knat = [actp.tile([128, E], BF16, name=f"knat{s}") for s in range(SLICES)]
    vnat = [actp.tile([128, E], BF16, name=f"vnat{s}") for s in range(SLICES)]

    # --- Q projection: out Q^T tiles (m on partitions, r free) ---
    # For each n-chunk (512 of m), 4 m-tiles of 128 accumulate over kt.
    for nch in range(2):
        ps_q = [psp.tile([128, RPC], F32, tag="proj", name=f"psq{nch}_{i}") for i in range(4)]
        for m4 in range(4):
            mt = nch * 4 + m4
            nc.tensor.matmul(
                ps_q[m4],
                bq_s[:, mt * 128 : (mt + 1) * 128],
                ones,
                start=True,
                stop=False,
            )
        for kt in range(8):
            w = wpool.tile([128, RPC], BF16, tag="w")
            nc.sync.dma_start(
                w, d["wqt"][kt * 128 : (kt + 1) * 128, nch * 512 : (nch + 1) * 512]
            )
            for m4 in range(4):
                nc.tensor.matmul(
                    ps_q[m4],
                    w[:, m4 * 128 : (m4 + 1) * 128],
                    xt_sb[kt],
                    start=False,
                    stop=(kt == 7),
                )
        # reorganize into qTi: psum (m=(c1,dd) x r=(s,r1)) -> qTi[dd, s, r1, c1]
        for m4 in range(4):
            mt = nch * 4 + m4
            for h in range(2):
                c1 = 2 * mt + h
                src = ps_q[m4][h * 64 : (h + 1) * 64, :].rearrange(
                    "d (s r) -> d s r", s=SLICES
                )
                dst = qTi_v[:, :, :, c1]
                if h == 0:
                    nc.vector.tensor_copy(dst, src)
                else:
                    nc.scalar.copy(dst, src)

    # --- K and V projections: natural orientation (r on partitions) ---
    for name_w, bias_t, dest in (("wkt", bk_s, knat), ("wvt", bv_s, vnat)):
        for nch in range(2):
            ps_kv = [psp.tile([128, RPC], F32, tag="proj", name=f"pskv{name_w}{nch}_{i}") for i in range(4)]
            for rt in range(4):
                nc.tensor.matmul(
                    ps_kv[rt],
                    ones[:, :128],
                    bias_t[:, nch * 512 : (nch + 1) * 512],
                    start=True,
                    stop=False,
                )
            for kt in range(8):
                w = wpool.tile([128, RPC], BF16, tag="w")
                nc.sync.dma_start(
                    w,
                    d[name_w][kt * 128 : (kt + 1) * 128, nch * 512 : (nch + 1) * 512],
                )
                for rt in range(4):
                    nc.tensor.matmul(
                        ps_kv[rt],
                        xt_sb[kt][:, rt * 128 : (rt + 1) * 128],
                        w,
                        start=False,
                        stop=(kt == 7),
                    )
            for rt in range(4):
                dst = dest[rt][:, nch * 512 : (nch + 1) * 512]
                if rt % 2 == 0:
                    nc.vector.tensor_copy(dst, ps_kv[rt])
                else:
                    nc.scalar.copy(dst, ps_kv[rt])

    # --- attention: M_s = sum_c2 Kc2^T Vc2 ; O_s^T = M_s^T Q_s^T ---
    for s in range(SLICES):
        ps_m = pso.tile([64, 512], F32, tag="pot", name=f"psm{s}")
        for c2 in range(16):
            nc.tensor.matmul(
                ps_m[:, 0:64],
                knat[s][:, c2 * 64 : (c2 + 1) * 64],
                vnat[s][:, c2 * 64 : (c2 + 1) * 64],
                start=(c2 == 0),
                stop=(c2 == 15),
            )
        m_sb = smallp.tile([64, 64], BF16, tag="msb")
        nc.vector.tensor_copy(m_sb, ps_m[:, 0:64])

        ot_sb = opool.tile([64, SEQ], F32, tag="ot")
        for g in range(4):
            ps_ot = pso.tile([64, 512], F32, tag="pot")
            nc.tensor.matmul(
                ps_ot,
                m_sb,
                qTi[:, (s * 4 + g) * 512 : (s * 4 + g + 1) * 512],
                start=True,
                stop=True,
            )
            dst = ot_sb[:, g * 512 : (g + 1) * 512]
            if g % 2 == 0:
                nc.vector.tensor_copy(dst, ps_ot)
            else:
                nc.scalar.copy(dst, ps_ot)
        nc.sync.dma_start(d["out"][s], ot_sb)


def _build():
    global _BUILT
    if _BUILT is not None:
        return _BUILT
    nc = bacc.Bacc(trn_type="TRN2", target_bir_lowering=False, debug=False)
    d = {
        "xt": nc.dram_tensor("xt", [E, RPC], BF16, kind="ExternalInput").ap(),
        "wqt": nc.dram_tensor("wqt", [E, E], BF16, kind="ExternalInput").ap(),
        "wkt": nc.dram_tensor("wkt", [E, E], BF16, kind="ExternalInput").ap(),
        "wvt": nc.dram_tensor("wvt", [E, E], BF16, kind="ExternalInput").ap(),
        "bq": nc.dram_tensor("bq", [1, E], BF16, kind="ExternalInput").ap(),
        "bk": nc.dram_tensor("bk", [1, E], BF16, kind="ExternalInput").ap(),
        "bv": nc.dram_tensor("bv", [1, E], BF16, kind="ExternalInput").ap(),
        "out": nc.dram_tensor("out", [SLICES, D, SEQ], F32, kind="ExternalOutput").ap(),
    }
    with tile.TileContext(nc) as tc:
        with ExitStack() as ctx:
            _body(ctx, tc, d)
    nc.compile()
    _BUILT = nc
    return nc


def kernel(x, Wq, bq, Wk, bk, Wv, bv):
    global LAST_RESULT
    x = np.asarray(x, dtype=np.float32)
    scale = np.float32(1.0 / 8.0)

    import ml_dtypes

    bf16 = ml_dtypes.bfloat16
    wqt = (np.ascontiguousarray(np.asarray(Wq, np.float32).T) * scale).astype(bf16)
    wkt = np.ascontiguousarray(np.asarray(Wk, np.float32).T).astype(bf16)
    wvt = np.ascontiguousarray(np.asarray(Wv, np.float32).T).astype(bf16)
    bq2 = (np.asarray(bq, np.float32) * scale).reshape(1, E).astype(bf16)
    bk2 = np.asarray(bk, np.float32).reshape(1, E).astype(bf16)
    bv2 = np.asarray(bv, np.float32).reshape(1, E).astype(bf16)

    x2 = x.reshape(-1, E)  # (4096, 1024)
    in_maps = []
    for c in range(NCORES):
        xt = np.ascontiguousarray(x2[c * RPC : (c + 1) * RPC].T).astype(bf16)
        in_maps.append(
            {
                "xt": xt,
                "wqt": wqt,
                "wkt": wkt,
                "wvt": wvt,
                "bq": bq2,
                "bk": bk2,
                "bv": bv2,
            }
        )

    nc = _build()
    res = run_bass_kernel_spmd(nc, in_maps, core_ids=list(range(NCORES)), trace=TRACE)
    LAST_RESULT = res

    out = np.empty((NCORES * SLICES, SEQ, D), dtype=np.float32)
    for c in range(NCORES):
        oc = res.results[c]["out"]  # (4, 64, 2048)
        for s in range(SLICES):
            out[c * SLICES + s] = oc[s].T
    return out


# revision 10
# speedup vs baseline: 1.1748x; 1.0387x over previous
"""Trainium2 Bass kernel for nn_MHAttention_60215441490286.

Reference computation (per batch b=2, seq s=2048, E=1024, H=16, D=64):
    q = x @ Wq.T + bq   (same k, v)
    q -> reshape(b, s, h, d) -> reshape(-1, s, d)      # RAW reshape, no transpose
    qk = einsum('bqd,bkd->bqk', q, k) / sqrt(d)
    out = einsum('bqk,bkd->bqd', qk, v)                # NO softmax

Key structure facts exploited here:
  * The raw reshape means slice i of the (32, 2048, 64) tensors is just rows
    [128*i, 128*(i+1)) of the flat (4096, 1024) projection output, reshaped.
    So slice i depends only on 128 rows of x -> embarrassingly parallel over
    slices. 8 cores x 4 slices each, no collectives.
  * No softmax => O_i = (Q_i K_i^T) V_i = Q_i (K_i^T V_i). The inner
    M_i = K_i^T V_i is only 64x64, so the 2048x2048 score matrix is never
    materialized. Attention FLOPs drop ~32x; QKV projections dominate.
  * All matmuls run in bf16 (host-cast inputs, f32 PSUM accumulation).
  * scale 1/sqrt(64)=1/8 is folded into Wq/bq on the host.

Per-core layouts (core c owns x rows [512c, 512c+512) = slices 4c..4c+3):
  xt   (1024, 512)  = x rows transposed (e on partitions)
  wqt/wkt/wvt (1024,1024) = W.T (e on partitions)
  Q: computed as Q^T tiles (m on partitions, r free), then reorganized into
     qTi (64, 8192) interleaved layout: qTi[dd, s*2048 + r1*16 + c1]
     = Qraw[512c + s*128 + r1, 64*c1 + dd]  (this is Q_slice^T per slice)
  K,V: computed in natural orientation knat/vnat[s] (128 rows, 1024 cols)
  M_s = sum_c2 Kc2^T @ Vc2 (64x64 in PSUM, 16 accumulating matmuls)
  O_s^T = M_s^T @ Q_s^T -> matmul(lhsT=M_s, rhs=qTi slice) (64, 2048)
  out DRAM per core: (4, 64, 2048) = O^T per slice; host transposes.
"""

import sys

if "/opt/trn_rl_repo" not in sys.path:
    sys.path.insert(0, "/opt/trn_rl_repo")

from contextlib import ExitStack

import numpy as np

import concourse.bass as bass
import concourse.tile as tile
from concourse import bacc
from concourse import mybir
from concourse.bass_utils import run_bass_kernel_spmd

F32 = mybir.dt.float32
F32R = mybir.dt.float32r
BF16 = mybir.dt.bfloat16

NCORES = 8
E = 1024
D = 64
H = 16
RPC = 512  # x rows per core
SLICES = 4  # slices per core (32 total / 8 cores)
SEQ = 2048  # positions per slice (128 rows x 16 chunks)

# test.py pokes these
TRACE = False
LAST_RESULT = None
_BUILT = None


def _body(ctx: ExitStack, tc: tile.TileContext, d):
    nc = tc.nc

    const = ctx.enter_context(tc.tile_pool(name="const", bufs=1))
    xpool = ctx.enter_context(tc.tile_pool(name="xp", bufs=1))
    wpool = ctx.enter_context(tc.tile_pool(name="wp", bufs=12))
    actp = ctx.enter_context(tc.tile_pool(name="act", bufs=1))
    smallp = ctx.enter_context(tc.tile_pool(name="small", bufs=2))
    opool = ctx.enter_context(tc.tile_pool(name="osb", bufs=2))
    psp = ctx.enter_context(tc.tile_pool(name="psp", bufs=6, space="PSUM"))
    pso = ctx.enter_context(tc.tile_pool(name="pso", bufs=2, space="PSUM"))

    # --- constants ---
    ones = const.tile([1, RPC], BF16, name="ones")
    nc.any.memset(ones, 1.0)
    bq_s = const.tile([1, E], BF16, name="bq_s")
    nc.sync.dma_start(bq_s, d["bq"])
    bk_s = const.tile([1, E], BF16, name="bk_s")
    nc.sync.dma_start(bk_s, d["bk"])
    bv_s = const.tile([1, E], BF16, name="bv_s")
    nc.sync.dma_start(bv_s, d["bv"])

    # --- x^T tiles: 8 x (128, 512), DMAs issued lazily in the first kt loop ---
    xt_sb = [xpool.tile([128, RPC], BF16, name=f"xt{kt}") for kt in range(8)]

    # --- activation destinations ---
    # qTi[dd, s*2048 + r1*16 + c1] ; view (64, 4, 128, 16)
    qTi = actp.tile([64, SLICES * SEQ], BF16, name="qTi")
    qTi_v = qTi.rearrange("d (s r c) -> d s r c", s=SLICES, r=128, c=16)
    knat = [actp.tile([128, E], BF16, name=f"knat{s}") for s in range(SLICES)]
    vnat = [actp.tile([128, E], BF16, name=f"vnat{s}") for s in range(SLICES)]

    # --- Q projection: out Q^T tiles (m on partitions, r free) ---
    # For each n-chunk (512 of m), 4 m-tiles of 128 accumulate over kt.
    for nch in range(2):
        ps_q = [psp.tile([128, RPC], F32, tag="proj", name=f"psq{nch}_{i}") for i in range(4)]
        for m4 in range(4):
            mt = nch * 4 + m4
            nc.tensor.matmul(
                ps_q[m4],
                bq_s[:, mt * 128 : (mt + 1) * 128],
                ones,
                start=True,
                stop=False,
            )
        for kt in range(8):
            if nch == 0:
                nc.sync.dma_start(xt_sb[kt], d["xt"][kt * 128 : (kt + 1) * 128, :])
            w = wpool.tile([128, RPC], BF16, tag="w")
            nc.sync.dma_start(
                w, d["wqt"][kt * 128 : (kt + 1) * 128, nch * 512 : (nch + 1) * 512]
            )
            for m4 in range(4):
                nc.tensor.matmul(
                    ps_q[m4],
                    w[:, m4 * 128 : (m4 + 1) * 128],
                    xt_sb[kt],
                    start=False,
                    stop=(kt == 7),
                )
        # reorganize into qTi: psum (m=(c1,dd) x r=(s,r1)) -> qTi[dd, s, r1, c1]
        for m4 in range(4):
            mt = nch * 4 + m4
            for h in range(2):
                c1 = 2 * mt + h
                src = ps_q[m4][h * 64 : (h + 1) * 64, :].rearrange(
                    "d (s r) -> d s r", s=SLICES
                )
                dst = qTi_v[:, :, :, c1]
                if h == 0:
                    nc.vector.tensor_copy(dst, src)
                else:
                    nc.scalar.copy(dst, src)

    # --- K and V projections: natural orientation (r on partitions) ---
    for name_w, bias_t, dest in (("wkt", bk_s, knat), ("wvt", bv_s, vnat)):
        for nch in range(2):
            ps_kv = [psp.tile([128, RPC], F32, tag="proj", name=f"pskv{name_w}{nch}_{i}") for i in range(4)]
            for rt in range(4):
                nc.tensor.matmul(
                    ps_kv[rt],
                    ones[:, :128],
                    bias_t[:, nch * 512 : (nch + 1) * 512],
                    start=True,
                    stop=False,
                )
            for kt in range(8):
                w = wpool.tile([128, RPC], BF16, tag="w")
                nc.sync.dma_start(
                    w,
                    d[name_w][kt * 128 : (kt + 1) * 128, nch * 512 : (nch + 1) * 512],
                )
                for rt in range(4):
                    nc.tensor.matmul(
                        ps_kv[rt],
                        xt_sb[kt][:, rt * 128 : (rt + 1) * 128],
                        w,
                        start=False,
                        stop=(kt == 7),
                    )
            for rt in range(4):
                dst = dest[rt][:, nch * 512 : (nch + 1) * 512]
                if rt % 2 == 0:
                    nc.vector.tensor_copy(dst, ps_kv[rt])
                else:
                    nc.scalar.copy(dst, ps_kv[rt])

    # --- attention: M_s = sum_c2 Kc2^T Vc2 ; O_s^T = M_s^T Q_s^T ---
    for s in range(SLICES):
        ps_m = pso.tile([64, 512], F32, tag="pot", name=f"psm{s}")
        for c2 in range(16):
            nc.tensor.matmul(
                ps_m[:, 0:64],
                knat[s][:, c2 * 64 : (c2 + 1) * 64],
                vnat[s][:, c2 * 64 : (c2 + 1) * 64],
                start=(c2 == 0),
                stop=(c2 == 15),
            )
        m_sb = smallp.tile([64, 64], BF16, tag="msb")
        nc.vector.tensor_copy(m_sb, ps_m[:, 0:64])

        ot_sb = opool.tile([64, SEQ], F32, tag="ot")
        for g in range(4):
            ps_ot = pso.tile([64, 512], F32, tag="pot")
            nc.tensor.matmul(
                ps_ot,
                m_sb,
                qTi[:, (s * 4 + g) * 512 : (s * 4 + g + 1) * 512],
                start=True,
                stop=True,
            )
            dst = ot_sb[:, g * 512 : (g + 1) * 512]
            if g % 2 == 0:
                nc.vector.tensor_copy(dst, ps_ot)
            else:
                nc.scalar.copy(dst, ps_ot)
        nc.sync.dma_start(d["out"][s], ot_sb)


def _build():
    global _BUILT
    if _BUILT is not None:
        return _BUILT
    nc = bacc.Bacc(trn_type="TRN2", target_bir_lowering=False, debug=False)
    d = {
        "xt": nc.dram_tensor("xt", [E, RPC], BF16, kind="ExternalInput").ap(),
        "wqt": nc.dram_tensor("wqt", [E, E], BF16, kind="ExternalInput").ap(),
        "wkt": nc.dram_tensor("wkt", [E, E], BF16, kind="ExternalInput").ap(),
        "wvt": nc.dram_tensor("wvt", [E, E], BF16, kind="ExternalInput").ap(),
        "bq": nc.dram_tensor("bq", [1, E], BF16, kind="ExternalInput").ap(),
        "bk": nc.dram_tensor("bk", [1, E], BF16, kind="ExternalInput").ap(),
        "bv": nc.dram_tensor("bv", [1, E], BF16, kind="ExternalInput").ap(),
        "out": nc.dram_tensor("out", [SLICES, D, SEQ], F32, kind="ExternalOutput").ap(),
    }
    with tile.TileContext(nc) as tc:
        with ExitStack() as ctx:
            _body(ctx, tc, d)
    nc.compile()
    _BUILT = nc
    return nc


def kernel(x, Wq, bq, Wk, bk, Wv, bv):
    global LAST_RESULT
    x = np.asarray(x, dtype=np.float32)
    scale = np.float32(1.0 / 8.0)

    import ml_dtypes

    bf16 = ml_dtypes.bfloat16
    wqt = (np.ascontiguousarray(np.asarray(Wq, np.float32).T) * scale).astype(bf16)
    wkt = np.ascontiguousarray(np.asarray(Wk, np.float32).T).astype(bf16)
    wvt = np.ascontiguousarray(np.asarray(Wv, np.float32).T).astype(bf16)
    bq2 = (np.asarray(bq, np.float32) * scale).reshape(1, E).astype(bf16)
    bk2 = np.asarray(bk, np.float32).reshape(1, E).astype(bf16)
    bv2 = np.asarray(bv, np.float32).reshape(1, E).astype(bf16)

    x2 = x.reshape(-1, E)  # (4096, 1024)
    in_maps = []
    for c in range(NCORES):
        xt = np.ascontiguousarray(x2[c * RPC : (c + 1) * RPC].T).astype(bf16)
        in_maps.append(
            {
                "xt": xt,
                "wqt": wqt,
                "wkt": wkt,
                "wvt": wvt,
                "bq": bq2,
                "bk": bk2,
                "bv": bv2,
            }
        )

    nc = _build()
    res = run_bass_kernel_spmd(nc, in_maps, core_ids=list(range(NCORES)), trace=TRACE)
    LAST_RESULT = res

    out = np.empty((NCORES * SLICES, SEQ, D), dtype=np.float32)
    for c in range(NCORES):
        oc = res.results[c]["out"]  # (4, 64, 2048)
        for s in range(SLICES):
            out[c * SLICES + s] = oc[s].T
    return out
